# revision 29
# baseline (speedup 1.0000x reference)
"""BatchSRU Trainium2 kernel (nn_BatchSRU_27556510171508) — v5.

Full inputs: x (2048, 8, 128, 16) f32, W (16, 128, 384), b (16, 256).
Sharding: data-parallel over the inner batch B=8 -> one batch row per
NeuronCore (zero cross-core communication); W/b replicated.

v5 = v3 pipeline (all elementwise work on DVE; GpSimd measured ~3x
slower on HW and poisons the scan->highway chain) plus:
  - t = c - x^T computed as instance PAIRS ([128, 2*LC] ops) right
    after the pair's second scan — halves the op count and bubbles
  - u = r*t computed as ONE whole-group op [128, 4*LC], deferred two
    instances so it never sits between two scans on the in-order DVE
  - per-instance carry-save ACT copies eliminated: each chunk's scan
    chains initial= straight to the previous chunk's cw[:, j, -1:]
    (cw tags are per-group so the previous chunk's tile stays alive)
  - (v8) highway merge h = x + u^T moved OFF the DVE: an identity
    matmul copies the f32 x slice into the PSUM piece (start=True), the
    out-transposes accumulate u^T on top (start=False), and an ACT copy
    evacuates the sum. DVE bf16 ops measured ~564ns/[128,512] on HW (no
    2x packed mode), so shedding the 658ns back-add cut the DVE wall
    from ~3.0us to ~2.4us per instance. HW: 233.5us -> 220.0us.
"""
import numpy as np
from contextlib import ExitStack

import concourse.bacc as bacc
import concourse.tile as tile
from concourse import mybir
from concourse.masks import make_identity

F32 = mybir.dt.float32
BF16 = mybir.dt.bfloat16
AL = mybir.AluOpType
AF = mybir.ActivationFunctionType

L, B, D, NB = 2048, 8, 128, 16
LC = 512
NCH = L // LC
QNB = 4
NQ = NB // QNB
NLS = LC // 128

N_CORES = 8


def _build(repeat: int = 1, unroll=False, carry_elim=True, ugroup=True, tpair=True, psplit=False, u_pool=False, t_pool=False):
    nc = bacc.Bacc("TRN2")
    x = nc.dram_tensor("x", [L, NB, D], F32, kind="ExternalInput")
    w = nc.dram_tensor("w", [NB, D, 3 * D], F32, kind="ExternalInput")
    bb = nc.dram_tensor("bb", [NB, 2 * D], F32, kind="ExternalInput")
    out = nc.dram_tensor("out", [L, NB, D], F32, kind="ExternalOutput")

    with tile.TileContext(nc) as tc, ExitStack() as ctx:
        const = ctx.enter_context(tc.tile_pool(name="const", bufs=1))

        ident = const.tile([128, 128], F32)
        make_identity(nc, ident)
        identb = const.tile([128, 128], BF16)
        make_identity(nc, identb)
        wr = const.tile([128, NB, 3 * D], BF16)
        bsb = const.tile([128, NB, 2], F32)
        nc.scalar.dma_start(out=bsb, in_=bb.rearrange("n (g d) -> d n g", d=128))
        carry = const.tile([128, NB], BF16)
        nc.vector.memset(carry, 0.0)

        with tc.tile_pool(name="wtmp_pool", bufs=1) as wtmp_pool:
            wtmp = wtmp_pool.tile([128, NB, 3 * D], F32)
            for wi in range(4):
                sl = slice(wi * 4, (wi + 1) * 4)
                nc.scalar.dma_start(out=wtmp[:, sl], in_=w.transpose([1, 0, 2])[:, sl])
                nc.vector.tensor_copy(wr[:, sl], wtmp[:, sl])

        xpool = ctx.enter_context(tc.tile_pool(name="xpool", bufs=2))
        sb = ctx.enter_context(tc.tile_pool(name="sb", bufs=2))
        if psplit:
            px = ctx.enter_context(tc.tile_pool(name="px", bufs=3, space="PSUM"))
            pfr = ctx.enter_context(tc.tile_pool(name="pfr", bufs=2, space="PSUM"))
            ph = ctx.enter_context(tc.tile_pool(name="ph", bufs=1, space="PSUM"))
        else:
            pu = ctx.enter_context(tc.tile_pool(name="pu", bufs=2, space="PSUM"))
            ph = ctx.enter_context(tc.tile_pool(name="ph", bufs=2, space="PSUM"))

        import contextlib

        pending = []
        u_pend = []
        dma_left = {}

        def piece_mms(p):
            # h = x + u^T assembled IN PSUM: an identity matmul copies the
            # f32 x slice in (PE has slack), the four out-transposes
            # accumulate u^T on top (start=False) — no DVE back-add.
            ready, rw, qq, ls, xts_t, plc, _ = p
            hps = ph.tile([128, QNB * 128], F32, tag="ph", name="hps")
            xv = xts_t[:, qq * QNB * D : (qq + 1) * QNB * D]
            nc.tensor.matmul(hps, ident, xv, start=True, stop=False)
            for j in range(QNB):
                nc.tensor.matmul(
                    hps[:, j * 128 : (j + 1) * 128],
                    rw[:, j, ls * 128 : ls * 128 + 128],
                    identb,
                    start=False,
                    stop=(j == QNB - 1),
                )
            p[6] = hps

        def piece_evac(p):
            # ACT copy evacuates h; emitted a full instance after the
            # matmuls so it never makes the next sigmoid wait on them
            ready, rw, qq, ls, xts_t, plc, hps = p
            xv = xts_t[:, qq * QNB * D : (qq + 1) * QNB * D]
            nc.scalar.copy(xv, hps)
            left, dlc, dls = dma_left[id(xts_t)]
            left -= 1
            dma_left[id(xts_t)] = (left, dlc, dls)
            if left == 0:
                l0 = dlc * LC + dls * 128
                nc.sync.dma_start(
                    out=out[l0 : l0 + 128].rearrange("l n d -> l (n d)"),
                    in_=xts_t,
                )

        def drain_mms(gidx, lag=6):
            # strictly head-ordered: only the head piece, and only after
            # the previous piece fully evacuated (no forward scanning)
            if pending and pending[0][6] is None and gidx >= pending[0][0] + lag:
                piece_mms(pending[0])

        def drain_pending(gidx, lag=6):
            if pending and pending[0][6] is not None and gidx >= pending[0][0] + lag:
                p = pending.pop(0)
                piece_evac(p)

        def drain_u(gidx):
            while u_pend and gidx >= u_pend[0][0]:
                _, uw_, rw_, tw_ = u_pend.pop(0)
                eng = nc.gpsimd if u_pool else nc.vector
                eng.tensor_tensor(uw_[:, :], rw_[:, :], tw_[:, :], AL.mult)

        cw_prev = {}
        n_unroll = repeat if unroll else 1
        loop_cm = (
            tc.For_i(0, repeat) if repeat > 1 and not unroll
            else contextlib.nullcontext()
        )
        with loop_cm:
         for lc0 in range(NCH * n_unroll):
            lc = lc0 % NCH
            xts = []
            for ls in range(NLS):
                xt_in = xpool.tile([128, D * NB], F32, tag=f"X{ls}")
                l0 = lc * LC + ls * 128
                nc.sync.dma_start(
                    out=xt_in, in_=x[l0 : l0 + 128].rearrange("l n d -> l (n d)")
                )
                xts.append(xt_in)
                dma_left[id(xt_in)] = (NQ, lc, ls)

            def gtiles(q):
                s = q % 2
                xTw = sb.tile([128, QNB, LC], BF16, tag=f"xT{s}", name=f"xT{s}")
                fw = sb.tile([128, QNB, LC], BF16, tag=f"f{s}", name=f"f{s}")
                rw = sb.tile([128, QNB, LC], BF16, tag=f"r{s}", name=f"r{s}")
                gw = sb.tile([128, QNB, LC], BF16, tag=f"g{s}", name=f"g{s}")
                cw = sb.tile(
                    [128, QNB, LC], BF16,
                    tag=(f"c{q}" if carry_elim else f"c{s}"), name="cw",
                )
                tw = sb.tile([128, QNB, LC], BF16, tag=f"t{s}", name=f"t{s}")
                uw = sb.tile([128, QNB, LC], BF16, tag=f"u{s}", name=f"u{s}")
                return xTw, fw, rw, gw, cw, tw, uw

            def in_transpose(i, pui, xTw):
                j = i % QNB
                px_i = pui[0]
                for ls in range(NLS):
                    xg = xts[ls][:, i * D : (i + 1) * D]
                    nc.tensor.transpose(
                        px_i[:, ls * 128 : (ls + 1) * 128], xg, ident
                    )
                nc.scalar.copy(xTw[:, j], px_i)

            grp = {}
            pu_i = [None] * (NB + 1)

            def alloc_pu():
                if psplit:
                    return (
                        px.tile([128, LC], F32, tag="px", name="px"),
                        pfr.tile([128, 2, LC], F32, tag="pfr", name="pfr"),
                    )
                t_ = pu.tile([128, 3, LC], F32, tag="pu", name="pu")
                return (t_[:, 0], t_[:, 1:3])

            grp[0] = gtiles(0)
            pu_i[0] = alloc_pu()
            in_transpose(0, pu_i[0], grp[0][0])

            for i in range(NB):
                q, j = i // QNB, i % QNB
                gidx = lc0 * NB + i
                drain_mms(gidx)
                xTw, fw, rw, gw, cw, tw, uw = grp[q]

                if i + 1 < NB:
                    qn = (i + 1) // QNB
                    if (i + 1) % QNB == 0:
                        grp[qn] = gtiles(qn)
                    pu_i[i + 1] = alloc_pu()
                    in_transpose(i + 1, pu_i[i + 1], grp[qn][0])

                px_i, pfr_i = pu_i[i]
                nc.tensor.matmul(
                    pfr_i[:, 0], wr[:, i, 128:256], xTw[:, j], start=True, stop=True
                )
                nc.tensor.matmul(
                    pfr_i[:, 1], wr[:, i, 256:384], xTw[:, j], start=True, stop=True
                )
                nc.tensor.matmul(
                    px_i, wr[:, i, 0:128], xTw[:, j], start=True, stop=True
                )
                nc.scalar.activation(
                    fw[:, j], pfr_i[:, 0], AF.Sigmoid, bias=bsb[:, i, 0:1], scale=1.0
                )
                nc.scalar.activation(
                    rw[:, j], pfr_i[:, 1], AF.Sigmoid, bias=bsb[:, i, 1:2], scale=1.0
                )
                nc.vector.scalar_tensor_tensor(
                    gw[:, j], fw[:, j], -1.0, px_i, AL.add, AL.mult
                )
                if carry_elim:
                    init = (
                        carry[:, i : i + 1]
                        if lc == 0
                        else cw_prev[q][:, j, LC - 1 : LC]
                    )
                else:
                    init = carry[:, i : i + 1]
                nc.vector.tensor_tensor_scan(
                    cw[:, j], fw[:, j], gw[:, j], init,
                    op0=AL.mult, op1=AL.subtract,
                )
                if not carry_elim:
                    nc.scalar.copy(carry[:, i : i + 1], cw[:, j, LC - 1 : LC])
                if t_pool:
                    nc.gpsimd.tensor_tensor(
                        tw[:, j], cw[:, j], xTw[:, j], AL.subtract
                    )
                elif tpair:
                    if j % 2 == 1:
                        nc.vector.tensor_tensor(
                            tw[:, j - 1 : j + 1], cw[:, j - 1 : j + 1],
                            xTw[:, j - 1 : j + 1], AL.subtract,
                        )
                else:
                    nc.vector.tensor_tensor(tw[:, j], cw[:, j], xTw[:, j], AL.subtract)
                if ugroup:
                    if j == QNB - 1:
                        u_pend.append((gidx + 2, uw, rw, tw))
                else:
                    nc.vector.tensor_tensor(uw[:, j], rw[:, j], tw[:, j], AL.mult)
                if j == QNB - 1:
                    cw_prev[q] = cw
                    for ls in range(NLS):
                        pending.append([gidx, uw, q, ls, xts[ls], lc, None])
                drain_u(gidx)
                drain_pending(gidx)

         drain_u(1 << 30)
         while pending:
            drain_mms(1 << 30)
            drain_pending(1 << 30)

    nc.finalize()
    return nc


_NC_CACHE = None


def _get_nc():
    global _NC_CACHE
    if _NC_CACHE is None:
        _NC_CACHE = _build()
    return _NC_CACHE


def make_in_maps(x, W, b):
    # per-core layout (L, NB, D): every on-device access pattern is then
    # contiguous (strided PE moving-operand reads are ~4x slower on HW)
    return [
        dict(x=np.ascontiguousarray(x[:, i].transpose(0, 2, 1)), w=W, bb=b)
        for i in range(N_CORES)
    ]


def assemble(outs):
    # outs: per-core (L, NB, D) -> full (L, B, D, NB)
    return np.stack([o.transpose(0, 2, 1) for o in outs], axis=1)


def kernel(x: np.ndarray, W: np.ndarray, b: np.ndarray) -> np.ndarray:
    assert x.shape == (L, B, D, NB) and W.shape == (NB, D, 3 * D)
    from concourse.bass_utils import run_bass_kernel_spmd

    nc = _get_nc()
    x = np.asarray(x, dtype=np.float32)
    W = np.asarray(W, dtype=np.float32)
    b = np.asarray(b, dtype=np.float32)
    in_maps = make_in_maps(x, W, b)
    results = run_bass_kernel_spmd(nc, in_maps, core_ids=list(range(N_CORES))).results
    return assemble([results[i]["out"] for i in range(N_CORES)])
